# revision 24
# baseline (speedup 1.0000x reference)
"""GQA attention (S=2048, D=4096, H=32, G=8, DH=128) on 8 trn2 cores.

Sharding: core i owns query heads [4i, 4i+4) and KV group i (column shards
of Wq/Wk/Wv) plus the matching ROW shard of Wo (rows [512i, 512i+512)).
After attention each core holds a normalized context slice ctxT_i
[512, 2048] (feature-major) in SBUF and computes a full partial output
out_i = ctx_i @ Wo_rows_i  [2048, 4096].  The host sums the 8 partials —
no on-device collective, which keeps the tensor engine dense and HAM-warm.

All activations are feature-major ([feature, seq]); all matmul operands
are bf16 (1 cycle/row, fp32 PSUM accumulation; bf16 streams with no
issue-rate penalty unlike fp16, and the 1024-wide moving operand lets two
heads share one matmul):
  qT_h = Wq_h^T @ x^T            (PE, accumulate over D tiles)
  RoPE via a signed half-swap permutation matmul + DVE muls
  s[t, 2q] = kT_tile.T @ qT_pair (one MM per HEAD-PAIR, scoresT layout)
  p    = exp(s/sqrt(DH) - 4)     (one ACT call per head-pair block)
  p   *= causal 0/1 mask         (DVE, diagonal blocks only)
  den  = sum_t p                 (DVE f32 partial sums + one ones-matmul)
  ctxT_pair = v_block.T @ p_pair (PE accumulate, one MM per pair)
  out  = ctxT_tile.T @ Wo_loc    (PE, N=1024, interleaved into the next
                                  chunk's softmax so the PE never idles)
Heads are processed in two passes of two (pass p: heads 2p, 2p+1) so the
paired score/ctx PSUM tiles + out-proj tiles fit in 8 PSUM banks.
"""

import math
import sys

if "/opt/trn_rl_repo" not in sys.path:
    sys.path.insert(0, "/opt/trn_rl_repo")

import numpy as np
import ml_dtypes

BF16 = ml_dtypes.bfloat16

S, D, H, G, DH = 2048, 4096, 32, 8, 128
N_CORES = 8
HPC = H // N_CORES          # query heads per core (4)
NHP = HPC // 2              # head pairs per core (2)
FPC = HPC * DH              # context features per core (512)
QC = 512                    # query chunk (per-head matmul free dim)
QC2 = 2 * QC                # paired free dim (1024)
NQC = S // QC               # 4
TB = 128                    # key block
NTB = S // TB               # 16
NKT = D // 128              # contraction tiles over D (32)
NJ = QC // TB               # key blocks per query chunk (4)
NOP = D // QC2              # out-proj column pair-chunks (4)
INV_SQRT_DH = 1.0 / math.sqrt(DH)
EXP_BIAS = -4.0             # keeps exp() outputs inside bf16 range

_CACHE = {}


def _build_program():
    import concourse.mybir as mybir
    import concourse.tile as tile
    from concourse import bacc

    f32 = mybir.dt.float32
    f16 = mybir.dt.bfloat16
    EXP = mybir.ActivationFunctionType.Exp

    nc = bacc.Bacc("TRN2", target_bir_lowering=False, debug=False,
                   num_devices=N_CORES)

    xT = nc.dram_tensor("xT", [128, NKT, S], f16, kind="ExternalInput")
    wq_d = nc.dram_tensor("wq", [128, NKT, FPC], f16, kind="ExternalInput")
    wk_d = nc.dram_tensor("wk", [128, NKT, DH], f16, kind="ExternalInput")
    wv_d = nc.dram_tensor("wv", [128, NKT, DH], f16, kind="ExternalInput")
    wo_d = nc.dram_tensor("wo", [FPC, D], f16, kind="ExternalInput")
    cosT_d = nc.dram_tensor("cosT", [DH, S], f32, kind="ExternalInput")
    sinT_d = nc.dram_tensor("sinT", [DH, S], f32, kind="ExternalInput")
    maskm_d = nc.dram_tensor("maskm", [NJ, TB, QC], f16, kind="ExternalInput")
    ident_d = nc.dram_tensor("ident", [TB, TB], f16, kind="ExternalInput")
    rotm_d = nc.dram_tensor("rotm", [DH, DH], f16, kind="ExternalInput")
    onesc_d = nc.dram_tensor("onesc", [TB, 1], f16, kind="ExternalInput")
    onesr_d = nc.dram_tensor("onesr", [1, DH], f16, kind="ExternalInput")
    out_d = nc.dram_tensor("out", [S, D], f16, kind="ExternalOutput")

    with tile.TileContext(nc) as tc:
        with tc.tile_pool(name="res", bufs=1) as res:
            # --- small resident constants (vector/gpsimd DMA queues, so the
            # sync queue starts streaming x/wq for compute immediately) ---
            ident_sb = res.tile([TB, TB], f16, tag="ident", name="ident_sb")
            nc.scalar.dma_start(out=ident_sb[:], in_=ident_d[:])
            rotm_sb = res.tile([DH, DH], f16, tag="rotm", name="rotm_sb")
            nc.scalar.dma_start(out=rotm_sb[:], in_=rotm_d[:])
            onesc_sb = res.tile([TB, 1], f16, tag="onesc", name="onesc_sb")
            onesr_sb = res.tile([1, DH], f16, tag="onesr", name="onesr_sb")
            ebias_sb = res.tile([128, 1], f32, tag="ebias", name="ebias_sb")
            nc.vector.memset(ebias_sb[:], EXP_BIAS)
            # causal mask duplicated per head-pair halves
            masks_sb = res.tile([TB, NJ, QC2], f16, tag="masks",
                                name="masks_sb")

            # --- persistent activations / weights ---
            # qT/kT/v are stored as PER-CHUNK tiles: attention on chunk 0
            # must not carry a (false, whole-tile) dependency on the later
            # chunks' rope writes.  qT is per head pair so one matmul can
            # stream both heads' queries (bf16 moving operand max is 1024).
            qT_sb = [[res.tile([128, 2, QC], f16, tag=f"qT{p}c{c}",
                               name=f"qT{p}c{c}_sb") for c in range(NQC)]
                     for p in range(NHP)]
            kT_sb = [res.tile([128, QC], f16, tag=f"kTc{c}",
                              name=f"kTc{c}_sb") for c in range(NQC)]
            v_sb = [res.tile([128, NJ, TB], f16, tag=f"vc{c}",
                             name=f"vc{c}_sb") for c in range(NQC)]
            wq_sb = res.tile([128, NKT, FPC], f16, tag="wq", name="wq_sb")
            wo_sb = res.tile([128, NJ, D], f16, tag="wo", name="wo_sb")

            # DMAs that aren't needed until attention time; deferred into
            # the chunk-1 x stream so they don't eat HBM bandwidth while
            # the first projections wait on x0/wq
            deferred = []
            deferred.append(lambda: nc.sync.dma_start(out=onesc_sb[:],
                                                      in_=onesc_d[:]))
            deferred.append(lambda: nc.sync.dma_start(out=onesr_sb[:],
                                                      in_=onesr_d[:]))
            for j in range(NJ):
                deferred.append(lambda j=j: nc.sync.dma_start(
                    out=masks_sb[:, j, 0:QC], in_=maskm_d[j]))
                deferred.append(lambda j=j: nc.sync.dma_start(
                    out=masks_sb[:, j, QC:QC2], in_=maskm_d[j]))
            for kt in range(NJ):
                deferred.append(lambda kt=kt: nc.sync.dma_start(
                    out=wo_sb[:, kt, :],
                    in_=wo_d[kt * 128:(kt + 1) * 128, :]))

            # normalized context, feature-major, per query chunk
            ctxT_sb = [res.tile([128, HPC * QC], f16, tag=f"ctxT{c % 2}",
                                name=f"ctxT{c}_sb") for c in range(NQC)]

            # ---- phase A: projections + RoPE ----
            with tc.tile_pool(name="pA", bufs=1) as pA, \
                 tc.tile_pool(name="psA", bufs=1, space="PSUM") as psA:
                wk_sb = pA.tile([128, NKT, DH], f16, tag="wk", name="wk_sb")
                wv_sb = pA.tile([128, NKT, DH], f16, tag="wv", name="wv_sb")

                def rope_copy(src_ps, eng):
                    qc_sb = pA.tile([128, QC], f16, tag="ropecp", bufs=6,
                                    name="qc_sb")
                    # psum->sbuf copies gate the next chunk's matmuls; split
                    # them over the scalar and vector queues
                    if eng == 0:
                        nc.scalar.copy(qc_sb[:], src_ps[:])
                    else:
                        nc.vector.tensor_copy(qc_sb[:], src_ps[:])
                    return qc_sb

                def rope_rot(qc_sb, sin_c, hold, tag="rot"):
                    rot_ps = psA.tile([128, QC], f32, tag=tag, bufs=1,
                                      name="rot_ps")
                    nc.tensor.matmul(rot_ps[:], rotm_sb[:], qc_sb[:],
                                     start=True, stop=True)
                    t1 = pA.tile([128, QC], f32, tag="ropet1", bufs=6,
                                 name="t1")
                    nc.vector.tensor_mul(t1[:], rot_ps[:], sin_c[:])
                    hold["t1"] = t1

                def rope_fin(qc_sb, hold, dst_ap, cos_c):
                    nc.vector.tensor_mul(dst_ap, qc_sb[:], cos_c[:])
                    nc.vector.tensor_add(dst_ap, dst_ap, hold["t1"])

                x0 = pA.tile([128, NKT, QC], f16, tag="x0", name="x0")
                pend_rope = []
                for c in range(NQC):
                    csl = slice(c * QC, (c + 1) * QC)
                    cos_c = pA.tile([DH, QC], f32, tag="cosc", bufs=2,
                                    name="cos_c")
                    sin_c = pA.tile([DH, QC], f32, tag="sinc", bufs=2,
                                    name="sin_c")
                    q_ps = []
                    for h in range(HPC):
                        qp = psA.tile([128, QC], f32, tag=f"pq{h}", bufs=1,
                                      name=f"q_ps{h}")
                        q_ps.append(qp)
                    k_ps = psA.tile([128, QC], f32, tag="pk", bufs=1,
                                    name="k_ps")
                    vT_ps = psA.tile([128, QC], f32, tag="pv", bufs=1,
                                     name="vT_ps")
                    if c == 0:
                        # chunk 0: keep x resident and sweep q first so the
                        # matmul order matches DMA arrival order (x block j
                        # then wq block j; wk/wv/cos/sin arrive last)
                        for k0, k1 in ((0, 2), (2, 8), (8, 16), (16, 24),
                                       (24, 32)):
                            ktsl = slice(k0, k1)
                            nc.sync.dma_start(out=x0[:, ktsl, :],
                                              in_=xT[:, ktsl, csl])
                            nc.sync.dma_start(out=wq_sb[:, ktsl, :],
                                              in_=wq_d[:, ktsl, :])
                        nc.sync.dma_start(out=wk_sb[:], in_=wk_d[:])
                        nc.sync.dma_start(out=wv_sb[:], in_=wv_d[:])
                        nc.sync.dma_start(out=cos_c[:], in_=cosT_d[:, csl])
                        nc.sync.dma_start(out=sin_c[:], in_=sinT_d[:, csl])
                        for kt in range(NKT):
                            st, sp = kt == 0, kt == NKT - 1
                            for h in range(HPC):
                                nc.tensor.matmul(
                                    q_ps[h][:],
                                    wq_sb[:, kt, h * DH:(h + 1) * DH],
                                    x0[:, kt, :], start=st, stop=sp)
                        for kt in range(NKT):
                            st, sp = kt == 0, kt == NKT - 1
                            nc.tensor.matmul(k_ps[:], wk_sb[:, kt, :],
                                             x0[:, kt, :], start=st, stop=sp)
                            nc.tensor.matmul(vT_ps[:], wv_sb[:, kt, :],
                                             x0[:, kt, :], start=st, stop=sp)
                    else:
                        for kt in range(NKT):
                            xt = pA.tile([128, QC], f16, tag="xt", bufs=4,
                                         name="xt")
                            nc.sync.dma_start(out=xt[:], in_=xT[:, kt, csl])
                            if pend_rope and kt % 3 == 2:
                                pend_rope.pop(0)()
                            if deferred and kt % 2 == 1:
                                deferred.pop(0)()
                            if kt == 8:
                                nc.sync.dma_start(out=cos_c[:],
                                                  in_=cosT_d[:, csl])
                                nc.sync.dma_start(out=sin_c[:],
                                                  in_=sinT_d[:, csl])
                            st, sp = kt == 0, kt == NKT - 1
                            for h in range(HPC):
                                nc.tensor.matmul(
                                    q_ps[h][:],
                                    wq_sb[:, kt, h * DH:(h + 1) * DH],
                                    xt[:], start=st, stop=sp)
                            nc.tensor.matmul(k_ps[:], wk_sb[:, kt, :], xt[:],
                                             start=st, stop=sp)
                            nc.tensor.matmul(vT_ps[:], wv_sb[:, kt, :], xt[:],
                                             start=st, stop=sp)
                    # chunk end: v transposes first (DVE-side copies) so
                    # the next chunk's k/v matmuls aren't gated behind the
                    # rope queues; the rot matmuls + muls are DEFERRED into
                    # the next chunk's matmul stream (the PE is in-order, so
                    # emitting them here would stall it on the DVE chain)
                    vts = pA.tile([128, QC], f16, tag="vts", bufs=2,
                                  name="vts")
                    nc.vector.tensor_copy(vts[:], vT_ps[:])
                    last = c == NQC - 1
                    kc = rope_copy(k_ps, 0)
                    qc_list = [rope_copy(q_ps[h], h % 2) for h in range(HPC)]
                    srcs = [kc] + qc_list
                    dsts = [kT_sb[c][:]] + [qT_sb[h // 2][c][:, h % 2, :]
                                            for h in range(HPC)]
                    # the final chunk's rots are emitted with no following
                    # matmul stream to hide the DVE chain; rotate them
                    # through the now-dead accumulator banks so all five
                    # issue back-to-back
                    tags = (["pk", "pq0", "pq1", "pq2", "pq3"] if last
                            else ["rot"] * 5)
                    for qc_sb, dst, tg in zip(srcs, dsts, tags):
                        hold = {}
                        pend_rope.append(
                            lambda q=qc_sb, s=sin_c, h=hold, t=tg:
                            rope_rot(q, s, h, t))
                        pend_rope.append(
                            lambda q=qc_sb, h=hold, d=dst, cc=cos_c:
                            rope_fin(q, h, d, cc))
                    for sb in range(NJ):
                        tr_ps = psA.tile([TB, TB], f16, tag="tr", bufs=1,
                                         name="tr_ps")
                        nc.tensor.transpose(tr_ps[:],
                                            vts[:, sb * TB:(sb + 1) * TB],
                                            ident_sb[:])
                        nc.vector.tensor_copy(v_sb[c][:, sb, :], tr_ps[:])
                        if last:
                            # final chunk: no next matmul stream to absorb
                            # the rope chain — interleave it here instead
                            for _ in range(2):
                                if pend_rope:
                                    pend_rope.pop(0)()

                for fn in pend_rope:
                    fn()
                del pend_rope[:]

            # ---- phases B (attention) + C (local out-proj), fused ----
            with tc.tile_pool(name="pB", bufs=1) as pB, \
                 tc.tile_pool(name="psB", bufs=1, space="PSUM") as psB, \
                 tc.tile_pool(name="pC", bufs=1) as pC, \
                 tc.tile_pool(name="psC", bufs=1, space="PSUM") as psC:

                def pass_b(qcn, hp, feeder, credit_per_tb, state, prologue):
                    """One head-pair pass of attention for query chunk qcn.

                    Returns (fin_a, fin_b): the denominator-reduction and
                    normalization emissions, deferred so they interleave with
                    the NEXT pass's score stream (keeps the PE busy through
                    the recip/broadcast latency chain).  `prologue` is the
                    previous pass's (fin_a, fin_b); fin_a is emitted before
                    the first block, fin_b after it."""
                    ntb = (qcn + 1) * NJ
                    ctx_ps = psB.tile([128, QC2], f32, tag="ctx", bufs=1,
                                      name="ctx_ps")
                    den_sb = pB.tile([128, QC2], f32, tag="den", bufs=2,
                                     name="den_sb")
                    dr_sb = pB.tile([128, QC2], f16, tag="dr", bufs=2,
                                    name="dr_sb")
                    p_list = []

                    def ctx_mm(tb, last):
                        pv = p_list[tb]
                        for i in range(2):
                            nc.tensor.matmul(
                                ctx_ps[:, i * QC:(i + 1) * QC],
                                v_sb[tb // NJ][:, tb % NJ, :],
                                pv[:, i * QC:(i + 1) * QC],
                                start=(tb == 0), stop=last)

                    if prologue is not None:
                        prologue[0]()
                    for tb in range(ntb):
                        j = tb - qcn * NJ
                        # the head pair's scores land in the two halves (two
                        # PSUM banks) of one tile so a single 1024-wide exp /
                        # mask / den op covers both heads
                        s_ps = psB.tile([128, QC2], f32, tag="s", bufs=2,
                                        name="s_ps")
                        tc_, tj = tb // NJ, tb % NJ
                        for i in range(2):
                            nc.tensor.matmul(
                                s_ps[:, i * QC:(i + 1) * QC],
                                kT_sb[tc_][:, tj * TB:(tj + 1) * TB],
                                qT_sb[hp][qcn][:, i, :],
                                start=True, stop=True)
                        p_sb = pB.tile([128, QC2], f16, tag="p",
                                       bufs=8, name="p_sb")
                        nc.scalar.activation(p_sb[:], s_ps[:], EXP,
                                             bias=ebias_sb[:],
                                             scale=INV_SQRT_DH)
                        if j >= 0:
                            nc.vector.tensor_mul(p_sb[:], p_sb[:],
                                                 masks_sb[:, j, :])
                        with nc.allow_low_precision(reason="softmax den"):
                            if tb == 0:
                                nc.vector.tensor_copy(den_sb[:], p_sb[:])
                            elif tb == ntb - 1:
                                nc.vector.tensor_add(dr_sb[:], den_sb[:],
                                                     p_sb[:])
                            else:
                                nc.vector.tensor_add(den_sb[:], den_sb[:],
                                                     p_sb[:])
                        p_list.append(p_sb)
                        if tb == 0 and prologue is not None:
                            prologue[1]()
                        state["credit"] += credit_per_tb
                        while state["credit"] >= 1.0:
                            state["credit"] -= 1.0
                            for _ in feeder:
                                break
                        # ctx matmuls lag one block so the exp has time to
                        # land without stalling the PE
                        if tb > 0:
                            ctx_mm(tb - 1, False)
                    ctx_mm(ntb - 1, True)
                    # free the ctx accumulator banks immediately (the next
                    # pass reuses them); normalization happens later in SBUF
                    ctxu_sb = pB.tile([128, QC2], f32, tag="ctxu", bufs=2,
                                      name="ctxu_sb")
                    nc.scalar.copy(ctxu_sb[:], ctx_ps[:])

                    hold = {}

                    def fin_a():
                        aux1 = psB.tile([128, QC2], f32, tag="s", bufs=2,
                                        name="aux1")
                        for i in range(2):
                            nc.tensor.matmul(aux1[:1, i * QC:(i + 1) * QC],
                                             onesc_sb[:],
                                             dr_sb[:, i * QC:(i + 1) * QC],
                                             start=True, stop=True)
                        recf = pB.tile([1, QC2], f32, tag="recf", bufs=2,
                                       name="recf")
                        nc.vector.reciprocal_approx_fast(out=recf[:],
                                                         in_=aux1[:1, :])
                        rech = pB.tile([1, QC2], f16, tag="rech", bufs=2,
                                       name="rech")
                        nc.vector.tensor_copy(rech[:], recf[:])
                        hold["rech"] = rech

                    def fin_b():
                        aux2 = psB.tile([128, QC2], f32, tag="s", bufs=2,
                                        name="aux2")
                        rech = hold["rech"]
                        for i in range(2):
                            nc.tensor.matmul(aux2[:, i * QC:(i + 1) * QC],
                                             onesr_sb[:],
                                             rech[:, i * QC:(i + 1) * QC],
                                             start=True, stop=True)
                        aux2s = pB.tile([128, QC2], f32, tag="aux2s", bufs=2,
                                        name="aux2s")
                        nc.scalar.copy(aux2s[:], aux2[:])
                        nc.vector.tensor_mul(
                            ctxT_sb[qcn][:, 2 * hp * QC:(2 * hp + 2) * QC],
                            ctxu_sb[:], aux2s[:])

                    return fin_a, fin_b

                def phase_c(qcn):
                    """Generator: out-proj of chunk qcn in (row-block, col)
                    groups of 4 accumulating matmuls + 1 copy."""
                    for qb in range(NJ):
                        o_sb = pC.tile([TB, D], f16, tag="osb", bufs=2,
                                       name="o_sb")
                        for n in range(2 * NOP):
                            nsl = slice(n * QC, (n + 1) * QC)
                            o_ps = psC.tile([TB, QC], f32, tag="o",
                                            bufs=2, name="o_ps")
                            for kt in range(NJ):
                                cb = kt * QC + qb * TB
                                nc.tensor.matmul(
                                    o_ps[:],
                                    ctxT_sb[qcn][:, cb:cb + TB],
                                    wo_sb[:, kt, nsl],
                                    start=(kt == 0),
                                    stop=(kt == NJ - 1))
                            nc.vector.tensor_copy(o_sb[:, nsl], o_ps[:])
                            yield True
                        qrow = qcn * QC + qb * TB
                        nc.sync.dma_start(out=out_d[qrow:qrow + TB, :],
                                          in_=o_sb[:])

                def drain(feeder):
                    for _ in feeder:
                        pass

                empty = iter(())
                ngrp = float(NJ * 2 * NOP)
                fin = None
                feeder = empty
                for qcn in range(NQC):
                    feeder = phase_c(qcn - 1) if qcn > 0 else empty
                    ntb = (qcn + 1) * NJ
                    credit = 0.7 * ngrp / (2 * ntb) if qcn > 0 else 0.0
                    state = {"credit": 0.0}
                    for hp in range(NHP):
                        fin = pass_b(qcn, hp, feeder, credit, state, fin)
                    if qcn < NQC - 1:
                        drain(feeder)
                # last pass's normalization, bridged by the held-back tail of
                # the previous chunk's out-proj
                fin[0]()
                drain(feeder)
                fin[1]()
                drain(phase_c(NQC - 1))
    nc.compile()
    return nc


def _host_consts():
    ident = np.eye(TB, dtype=BF16)
    rotm = np.zeros((DH, DH), dtype=BF16)
    half = DH // 2
    for d in range(half):
        rotm[d + half, d] = -1.0   # out[d] = -q[d+half]
        rotm[d, d + half] = 1.0    # out[d+half] = q[d]
    onesc = np.ones((TB, 1), dtype=BF16)
    onesr = np.ones((1, DH), dtype=BF16)
    maskm = np.zeros((NJ, TB, QC), dtype=BF16)
    tloc = np.arange(TB)[:, None]
    qloc = np.arange(QC)[None, :]
    for j in range(NJ):
        maskm[j] = (tloc + TB * j <= qloc).astype(BF16)
    return ident, rotm, onesc, onesr, maskm


def _swz(w):
    """[D, C] -> [128, NKT, C] with w[kt*128+p, c] at [p, kt, c]."""
    c = w.shape[1]
    return np.ascontiguousarray(
        w.reshape(NKT, 128, c).transpose(1, 0, 2)).astype(BF16)


def kernel(x, mask, cos, sin, Wq, Wk, Wv, Wo):
    from concourse.bass_utils import run_bass_kernel_spmd

    if "nc" not in _CACHE:
        _CACHE["nc"] = _build_program()
    nc = _CACHE["nc"]

    x = np.asarray(x, dtype=np.float32)
    cos = np.asarray(cos, dtype=np.float32)
    sin = np.asarray(sin, dtype=np.float32)
    Wq = np.asarray(Wq, dtype=np.float32)
    Wk = np.asarray(Wk, dtype=np.float32)
    Wv = np.asarray(Wv, dtype=np.float32)
    Wo = np.asarray(Wo, dtype=np.float32)

    xT = _swz(np.ascontiguousarray(x[0].T))          # [128, NKT, S]
    cosT = np.ascontiguousarray(cos.T)               # [DH, S]
    sinT = np.ascontiguousarray(sin.T)
    ident, rotm, onesc, onesr, maskm = _host_consts()

    in_maps = []
    for i in range(N_CORES):
        in_maps.append({
            "xT": xT,
            "wq": _swz(Wq[:, i * FPC:(i + 1) * FPC]),
            "wk": _swz(Wk[:, i * DH:(i + 1) * DH]),
            "wv": _swz(Wv[:, i * DH:(i + 1) * DH]),
            "wo": np.ascontiguousarray(Wo[i * FPC:(i + 1) * FPC, :]).astype(BF16),
            "cosT": cosT,
            "sinT": sinT,
            "maskm": maskm,
            "ident": ident,
            "rotm": rotm,
            "onesc": onesc,
            "onesr": onesr,
        })

    import os
    trace = bool(os.environ.get("BASS_TRACE"))
    res = run_bass_kernel_spmd(nc, in_maps, list(range(N_CORES)), trace=trace)
    _CACHE["last_exec_time_ns"] = res.exec_time_ns
    _CACHE["last_result"] = res

    out = np.zeros((S, D), dtype=np.float32)
    for i in range(N_CORES):
        out += res.results[i]["out"].astype(np.float32)
    return out[None]


# revision 25
# speedup vs baseline: 1.2129x; 1.2129x over previous
"""GQA attention (S=2048, D=4096, H=32, G=8, DH=128) on 8 trn2 cores.

Sharding: core i owns query heads [4i, 4i+4) and KV group i (column shards
of Wq/Wk/Wv) plus the matching ROW shard of Wo (rows [512i, 512i+512)).
After attention each core holds a normalized context slice ctxT_i
[512, 2048] (feature-major) in SBUF and computes a full partial output
out_i = ctx_i @ Wo_rows_i  [2048, 4096].  The host sums the 8 partials —
no on-device collective, which keeps the tensor engine dense and HAM-warm.

All activations are feature-major ([feature, seq]); all matmul operands
are bf16 (1 cycle/row, fp32 PSUM accumulation; bf16 streams with no
issue-rate penalty unlike fp16, and the 1024-wide moving operand lets two
heads share one matmul):
  qT_h = Wq_h^T @ x^T            (PE, accumulate over D tiles)
  RoPE via a signed half-swap permutation matmul + DVE muls
  s[t, 2q] = kT_tile.T @ qT_pair (one MM per HEAD-PAIR, scoresT layout)
  p    = exp(s/sqrt(DH) - 4)     (one ACT call per head-pair block)
  p   *= causal 0/1 mask         (DVE, diagonal blocks only)
  den  = sum_t p                 (DVE f32 partial sums + one ones-matmul)
  ctxT_pair = v_block.T @ p_pair (PE accumulate, one MM per pair)
  out  = ctxT_tile.T @ Wo_loc    (PE, N=1024, interleaved into the next
                                  chunk's softmax so the PE never idles)
Heads are processed in two passes of two (pass p: heads 2p, 2p+1) so the
paired score/ctx PSUM tiles + out-proj tiles fit in 8 PSUM banks.
"""

import math
import sys

if "/opt/trn_rl_repo" not in sys.path:
    sys.path.insert(0, "/opt/trn_rl_repo")

import numpy as np
import ml_dtypes

BF16 = ml_dtypes.bfloat16

S, D, H, G, DH = 2048, 4096, 32, 8, 128
N_CORES = 8
HPC = H // N_CORES          # query heads per core (4)
NHP = HPC // 2              # head pairs per core (2)
FPC = HPC * DH              # context features per core (512)
QC = 512                    # query chunk (per-head matmul free dim)
QC2 = 2 * QC                # paired free dim (1024)
NQC = S // QC               # 4
TB = 128                    # key block
NTB = S // TB               # 16
NKT = D // 128              # contraction tiles over D (32)
NJ = QC // TB               # key blocks per query chunk (4)
NOP = D // QC2              # out-proj column pair-chunks (4)
INV_SQRT_DH = 1.0 / math.sqrt(DH)
EXP_BIAS = -4.0             # keeps exp() outputs inside bf16 range

_CACHE = {}


def _build_program():
    import concourse.mybir as mybir
    import concourse.tile as tile
    from concourse import bacc

    f32 = mybir.dt.float32
    f16 = mybir.dt.bfloat16
    EXP = mybir.ActivationFunctionType.Exp

    nc = bacc.Bacc("TRN2", target_bir_lowering=False, debug=False,
                   num_devices=N_CORES)

    xT = nc.dram_tensor("xT", [128, NKT, S], f16, kind="ExternalInput")
    wq_d = nc.dram_tensor("wq", [128, NKT, FPC], f16, kind="ExternalInput")
    wk_d = nc.dram_tensor("wk", [128, NKT, DH], f16, kind="ExternalInput")
    wv_d = nc.dram_tensor("wv", [128, NKT, DH], f16, kind="ExternalInput")
    wo_d = nc.dram_tensor("wo", [FPC, D], f16, kind="ExternalInput")
    cosT_d = nc.dram_tensor("cosT", [DH, S], f32, kind="ExternalInput")
    sinT_d = nc.dram_tensor("sinT", [DH, S], f32, kind="ExternalInput")
    maskm_d = nc.dram_tensor("maskm", [NJ, TB, QC], f16, kind="ExternalInput")
    ident_d = nc.dram_tensor("ident", [TB, TB], f16, kind="ExternalInput")
    rotm_d = nc.dram_tensor("rotm", [DH, DH], f16, kind="ExternalInput")
    onesc_d = nc.dram_tensor("onesc", [TB, 1], f16, kind="ExternalInput")
    onesr_d = nc.dram_tensor("onesr", [1, DH], f16, kind="ExternalInput")
    out_d = nc.dram_tensor("out", [S, D], f16, kind="ExternalOutput")

    with tile.TileContext(nc) as tc:
        with tc.tile_pool(name="res", bufs=1) as res:
            # --- small resident constants (vector/gpsimd DMA queues, so the
            # sync queue starts streaming x/wq for compute immediately) ---
            ident_sb = res.tile([TB, TB], f16, tag="ident", name="ident_sb")
            nc.scalar.dma_start(out=ident_sb[:], in_=ident_d[:])
            rotm_sb = res.tile([DH, DH], f16, tag="rotm", name="rotm_sb")
            nc.scalar.dma_start(out=rotm_sb[:], in_=rotm_d[:])
            onesc_sb = res.tile([TB, 1], f16, tag="onesc", name="onesc_sb")
            onesr_sb = res.tile([1, DH], f16, tag="onesr", name="onesr_sb")
            ebias_sb = res.tile([128, 1], f32, tag="ebias", name="ebias_sb")
            nc.vector.memset(ebias_sb[:], EXP_BIAS)
            # causal mask duplicated per head-pair halves
            masks_sb = res.tile([TB, NJ, QC2], f16, tag="masks",
                                name="masks_sb")

            # --- persistent activations / weights ---
            # qT/kT/v are stored as PER-CHUNK tiles: attention on chunk 0
            # must not carry a (false, whole-tile) dependency on the later
            # chunks' rope writes.  qT is per head pair so one matmul can
            # stream both heads' queries (bf16 moving operand max is 1024).
            qT_sb = [[res.tile([128, 2, QC], f16, tag=f"qT{p}c{c}",
                               name=f"qT{p}c{c}_sb") for c in range(NQC)]
                     for p in range(NHP)]
            kT_sb = [res.tile([128, QC], f16, tag=f"kTc{c}",
                              name=f"kTc{c}_sb") for c in range(NQC)]
            v_sb = [res.tile([128, NJ, TB], f16, tag=f"vc{c}",
                             name=f"vc{c}_sb") for c in range(NQC)]
            wq_sb = res.tile([128, NKT, FPC], f16, tag="wq", name="wq_sb")
            wo_sb = res.tile([128, NJ, D], f16, tag="wo", name="wo_sb")

            # DMAs that aren't needed until attention time; deferred into
            # the chunk-1 x stream so they don't eat HBM bandwidth while
            # the first projections wait on x0/wq
            deferred = []
            deferred.append(lambda: nc.sync.dma_start(out=onesc_sb[:],
                                                      in_=onesc_d[:]))
            deferred.append(lambda: nc.sync.dma_start(out=onesr_sb[:],
                                                      in_=onesr_d[:]))
            for j in range(NJ):
                deferred.append(lambda j=j: nc.sync.dma_start(
                    out=masks_sb[:, j, 0:QC], in_=maskm_d[j]))
                deferred.append(lambda j=j: nc.sync.dma_start(
                    out=masks_sb[:, j, QC:QC2], in_=maskm_d[j]))
            for kt in range(NJ):
                deferred.append(lambda kt=kt: nc.sync.dma_start(
                    out=wo_sb[:, kt, :],
                    in_=wo_d[kt * 128:(kt + 1) * 128, :]))

            # normalized context, feature-major, per query chunk
            ctxT_sb = [res.tile([128, HPC * QC], f16, tag=f"ctxT{c % 2}",
                                name=f"ctxT{c}_sb") for c in range(NQC)]

            # ---- phase A: projections + RoPE ----
            with tc.tile_pool(name="pA", bufs=1) as pA, \
                 tc.tile_pool(name="psA", bufs=1, space="PSUM") as psA:
                wk_sb = pA.tile([128, NKT, DH], f16, tag="wk", name="wk_sb")
                wv_sb = pA.tile([128, NKT, DH], f16, tag="wv", name="wv_sb")

                def rope_copy(src_ps, eng):
                    qc_sb = pA.tile([128, QC], f16, tag="ropecp", bufs=6,
                                    name="qc_sb")
                    # psum->sbuf copies gate the next chunk's matmuls; split
                    # them over the scalar and vector queues
                    if eng == 0:
                        nc.scalar.copy(qc_sb[:], src_ps[:])
                    else:
                        nc.vector.tensor_copy(qc_sb[:], src_ps[:])
                    return qc_sb

                def rope_rot(qc_sb, sin_c, hold, tag="rot"):
                    rot_ps = psA.tile([128, QC], f32, tag=tag, bufs=1,
                                      name="rot_ps")
                    nc.tensor.matmul(rot_ps[:], rotm_sb[:], qc_sb[:],
                                     start=True, stop=True)
                    t1 = pA.tile([128, QC], f32, tag="ropet1", bufs=6,
                                 name="t1")
                    nc.vector.tensor_mul(t1[:], rot_ps[:], sin_c[:])
                    hold["t1"] = t1

                def rope_fin(qc_sb, hold, dst_ap, cos_c):
                    nc.vector.tensor_mul(dst_ap, qc_sb[:], cos_c[:])
                    nc.vector.tensor_add(dst_ap, dst_ap, hold["t1"])

                x0 = pA.tile([128, NKT, QC], f16, tag="x0", name="x0")
                pend_rope = []
                for c in range(NQC):
                    csl = slice(c * QC, (c + 1) * QC)
                    cos_c = pA.tile([DH, QC], f32, tag="cosc", bufs=2,
                                    name="cos_c")
                    sin_c = pA.tile([DH, QC], f32, tag="sinc", bufs=2,
                                    name="sin_c")
                    q_ps = []
                    for h in range(HPC):
                        qp = psA.tile([128, QC], f32, tag=f"pq{h}", bufs=1,
                                      name=f"q_ps{h}")
                        q_ps.append(qp)
                    k_ps = psA.tile([128, QC], f32, tag="pk", bufs=1,
                                    name="k_ps")
                    vT_ps = psA.tile([128, QC], f32, tag="pv", bufs=1,
                                     name="vT_ps")
                    if c == 0:
                        # chunk 0: keep x resident and sweep q first so the
                        # matmul order matches DMA arrival order (x block j
                        # then wq block j; wk/wv/cos/sin arrive last)
                        for k0, k1 in ((0, 2), (2, 8), (8, 16), (16, 24),
                                       (24, 32)):
                            ktsl = slice(k0, k1)
                            nc.sync.dma_start(out=x0[:, ktsl, :],
                                              in_=xT[:, ktsl, csl])
                            nc.sync.dma_start(out=wq_sb[:, ktsl, :],
                                              in_=wq_d[:, ktsl, :])
                        nc.sync.dma_start(out=wk_sb[:], in_=wk_d[:])
                        nc.sync.dma_start(out=wv_sb[:], in_=wv_d[:])
                        nc.sync.dma_start(out=cos_c[:], in_=cosT_d[:, csl])
                        nc.sync.dma_start(out=sin_c[:], in_=sinT_d[:, csl])
                        for kt in range(NKT):
                            st, sp = kt == 0, kt == NKT - 1
                            for h in range(HPC):
                                nc.tensor.matmul(
                                    q_ps[h][:],
                                    wq_sb[:, kt, h * DH:(h + 1) * DH],
                                    x0[:, kt, :], start=st, stop=sp)
                        for kt in range(NKT):
                            st, sp = kt == 0, kt == NKT - 1
                            nc.tensor.matmul(k_ps[:], wk_sb[:, kt, :],
                                             x0[:, kt, :], start=st, stop=sp)
                            nc.tensor.matmul(vT_ps[:], wv_sb[:, kt, :],
                                             x0[:, kt, :], start=st, stop=sp)
                    else:
                        for kt in range(NKT):
                            xt = pA.tile([128, QC], f16, tag="xt", bufs=4,
                                         name="xt")
                            nc.sync.dma_start(out=xt[:], in_=xT[:, kt, csl])
                            if pend_rope and kt % 3 == 2:
                                pend_rope.pop(0)()
                            if deferred and kt % 2 == 1:
                                deferred.pop(0)()
                            if kt == 8:
                                nc.sync.dma_start(out=cos_c[:],
                                                  in_=cosT_d[:, csl])
                                nc.sync.dma_start(out=sin_c[:],
                                                  in_=sinT_d[:, csl])
                            st, sp = kt == 0, kt == NKT - 1
                            for h in range(HPC):
                                nc.tensor.matmul(
                                    q_ps[h][:],
                                    wq_sb[:, kt, h * DH:(h + 1) * DH],
                                    xt[:], start=st, stop=sp)
                            nc.tensor.matmul(k_ps[:], wk_sb[:, kt, :], xt[:],
                                             start=st, stop=sp)
                            nc.tensor.matmul(vT_ps[:], wv_sb[:, kt, :], xt[:],
                                             start=st, stop=sp)
                    # chunk end: v transposes first (DVE-side copies) so
                    # the next chunk's k/v matmuls aren't gated behind the
                    # rope queues; the rot matmuls + muls are DEFERRED into
                    # the next chunk's matmul stream (the PE is in-order, so
                    # emitting them here would stall it on the DVE chain)
                    vts = pA.tile([128, QC], f16, tag="vts", bufs=2,
                                  name="vts")
                    nc.vector.tensor_copy(vts[:], vT_ps[:])
                    last = c == NQC - 1
                    kc = rope_copy(k_ps, 0)
                    qc_list = [rope_copy(q_ps[h], h % 2) for h in range(HPC)]
                    srcs = [kc] + qc_list
                    dsts = [kT_sb[c][:]] + [qT_sb[h // 2][c][:, h % 2, :]
                                            for h in range(HPC)]
                    for qc_sb, dst in zip(srcs, dsts):
                        hold = {}
                        pend_rope.append(
                            lambda q=qc_sb, s=sin_c, h=hold: rope_rot(q, s, h))
                        pend_rope.append(
                            lambda q=qc_sb, h=hold, d=dst, cc=cos_c:
                            rope_fin(q, h, d, cc))
                    for sb in range(NJ):
                        tr_ps = psA.tile([TB, TB], f16, tag="tr", bufs=1,
                                         name="tr_ps")
                        nc.tensor.transpose(tr_ps[:],
                                            vts[:, sb * TB:(sb + 1) * TB],
                                            ident_sb[:])
                        nc.vector.tensor_copy(v_sb[c][:, sb, :], tr_ps[:])
                        if last:
                            # final chunk: no next matmul stream to absorb
                            # the rope chain — interleave it here instead
                            for _ in range(2):
                                if pend_rope:
                                    pend_rope.pop(0)()

                for fn in pend_rope:
                    fn()
                del pend_rope[:]

            # ---- phases B (attention) + C (local out-proj), fused ----
            with tc.tile_pool(name="pB", bufs=1) as pB, \
                 tc.tile_pool(name="psB", bufs=1, space="PSUM") as psB, \
                 tc.tile_pool(name="pC", bufs=1) as pC, \
                 tc.tile_pool(name="psC", bufs=1, space="PSUM") as psC:

                def pass_b(qcn, hp, feeder, credit_per_tb, state, prologue):
                    """One head-pair pass of attention for query chunk qcn.

                    Returns (fin_a, fin_b): the denominator-reduction and
                    normalization emissions, deferred so they interleave with
                    the NEXT pass's score stream (keeps the PE busy through
                    the recip/broadcast latency chain).  `prologue` is the
                    previous pass's (fin_a, fin_b); fin_a is emitted before
                    the first block, fin_b after it."""
                    ntb = (qcn + 1) * NJ
                    ctx_ps = psB.tile([128, QC2], f32, tag="ctx", bufs=1,
                                      name="ctx_ps")
                    den_sb = pB.tile([128, QC2], f32, tag="den", bufs=2,
                                     name="den_sb")
                    dr_sb = pB.tile([128, QC2], f16, tag="dr", bufs=2,
                                    name="dr_sb")
                    p_list = []

                    def ctx_mm(tb, last):
                        pv = p_list[tb]
                        for i in range(2):
                            nc.tensor.matmul(
                                ctx_ps[:, i * QC:(i + 1) * QC],
                                v_sb[tb // NJ][:, tb % NJ, :],
                                pv[:, i * QC:(i + 1) * QC],
                                start=(tb == 0), stop=last)

                    if prologue is not None:
                        prologue[0]()
                    for tb in range(ntb):
                        j = tb - qcn * NJ
                        # the head pair's scores land in the two halves (two
                        # PSUM banks) of one tile so a single 1024-wide exp /
                        # mask / den op covers both heads
                        s_ps = psB.tile([128, QC2], f32, tag="s", bufs=2,
                                        name="s_ps")
                        tc_, tj = tb // NJ, tb % NJ
                        for i in range(2):
                            nc.tensor.matmul(
                                s_ps[:, i * QC:(i + 1) * QC],
                                kT_sb[tc_][:, tj * TB:(tj + 1) * TB],
                                qT_sb[hp][qcn][:, i, :],
                                start=True, stop=True)
                        p_sb = pB.tile([128, QC2], f16, tag="p",
                                       bufs=8, name="p_sb")
                        nc.scalar.activation(p_sb[:], s_ps[:], EXP,
                                             bias=ebias_sb[:],
                                             scale=INV_SQRT_DH)
                        if j >= 0:
                            nc.vector.tensor_mul(p_sb[:], p_sb[:],
                                                 masks_sb[:, j, :])
                        with nc.allow_low_precision(reason="softmax den"):
                            if tb == 0:
                                nc.vector.tensor_copy(den_sb[:], p_sb[:])
                            elif tb == ntb - 1:
                                nc.vector.tensor_add(dr_sb[:], den_sb[:],
                                                     p_sb[:])
                            else:
                                nc.vector.tensor_add(den_sb[:], den_sb[:],
                                                     p_sb[:])
                        p_list.append(p_sb)
                        if tb == 0 and prologue is not None:
                            prologue[1]()
                        state["credit"] += credit_per_tb
                        while state["credit"] >= 1.0:
                            state["credit"] -= 1.0
                            for _ in feeder:
                                break
                        # ctx matmuls lag one block so the exp has time to
                        # land without stalling the PE
                        if tb > 0:
                            ctx_mm(tb - 1, False)
                    ctx_mm(ntb - 1, True)
                    # free the ctx accumulator banks immediately (the next
                    # pass reuses them); normalization happens later in SBUF
                    ctxu_sb = pB.tile([128, QC2], f32, tag="ctxu", bufs=2,
                                      name="ctxu_sb")
                    nc.scalar.copy(ctxu_sb[:], ctx_ps[:])

                    hold = {}

                    def fin_a():
                        aux1 = psB.tile([128, QC2], f32, tag="s", bufs=2,
                                        name="aux1")
                        for i in range(2):
                            nc.tensor.matmul(aux1[:1, i * QC:(i + 1) * QC],
                                             onesc_sb[:],
                                             dr_sb[:, i * QC:(i + 1) * QC],
                                             start=True, stop=True)
                        recf = pB.tile([1, QC2], f32, tag="recf", bufs=2,
                                       name="recf")
                        nc.vector.reciprocal_approx_fast(out=recf[:],
                                                         in_=aux1[:1, :])
                        rech = pB.tile([1, QC2], f16, tag="rech", bufs=2,
                                       name="rech")
                        nc.vector.tensor_copy(rech[:], recf[:])
                        hold["rech"] = rech

                    def fin_b():
                        aux2 = psB.tile([128, QC2], f32, tag="s", bufs=2,
                                        name="aux2")
                        rech = hold["rech"]
                        for i in range(2):
                            nc.tensor.matmul(aux2[:, i * QC:(i + 1) * QC],
                                             onesr_sb[:],
                                             rech[:, i * QC:(i + 1) * QC],
                                             start=True, stop=True)
                        aux2s = pB.tile([128, QC2], f32, tag="aux2s", bufs=2,
                                        name="aux2s")
                        nc.scalar.copy(aux2s[:], aux2[:])
                        nc.vector.tensor_mul(
                            ctxT_sb[qcn][:, 2 * hp * QC:(2 * hp + 2) * QC],
                            ctxu_sb[:], aux2s[:])

                    return fin_a, fin_b

                def phase_c(qcn):
                    """Generator: out-proj of chunk qcn in (row-block, col)
                    groups of 4 accumulating matmuls + 1 copy."""
                    for qb in range(NJ):
                        o_sb = pC.tile([TB, D], f16, tag="osb", bufs=2,
                                       name="o_sb")
                        for n in range(2 * NOP):
                            nsl = slice(n * QC, (n + 1) * QC)
                            o_ps = psC.tile([TB, QC], f32, tag="o",
                                            bufs=2, name="o_ps")
                            for kt in range(NJ):
                                cb = kt * QC + qb * TB
                                nc.tensor.matmul(
                                    o_ps[:],
                                    ctxT_sb[qcn][:, cb:cb + TB],
                                    wo_sb[:, kt, nsl],
                                    start=(kt == 0),
                                    stop=(kt == NJ - 1))
                            nc.vector.tensor_copy(o_sb[:, nsl], o_ps[:])
                            yield True
                        qrow = qcn * QC + qb * TB
                        nc.sync.dma_start(out=out_d[qrow:qrow + TB, :],
                                          in_=o_sb[:])

                def drain(feeder):
                    for _ in feeder:
                        pass

                empty = iter(())
                ngrp = float(NJ * 2 * NOP)
                fin = None
                feeder = empty
                for qcn in range(NQC):
                    feeder = phase_c(qcn - 1) if qcn > 0 else empty
                    ntb = (qcn + 1) * NJ
                    credit = 0.7 * ngrp / (2 * ntb) if qcn > 0 else 0.0
                    state = {"credit": 0.0}
                    for hp in range(NHP):
                        fin = pass_b(qcn, hp, feeder, credit, state, fin)
                    if qcn < NQC - 1:
                        drain(feeder)
                # last pass's normalization, bridged by the held-back tail of
                # the previous chunk's out-proj
                fin[0]()
                drain(feeder)
                fin[1]()
                drain(phase_c(NQC - 1))
    nc.compile()
    return nc


def _host_consts():
    ident = np.eye(TB, dtype=BF16)
    rotm = np.zeros((DH, DH), dtype=BF16)
    half = DH // 2
    for d in range(half):
        rotm[d + half, d] = -1.0   # out[d] = -q[d+half]
        rotm[d, d + half] = 1.0    # out[d+half] = q[d]
    onesc = np.ones((TB, 1), dtype=BF16)
    onesr = np.ones((1, DH), dtype=BF16)
    maskm = np.zeros((NJ, TB, QC), dtype=BF16)
    tloc = np.arange(TB)[:, None]
    qloc = np.arange(QC)[None, :]
    for j in range(NJ):
        maskm[j] = (tloc + TB * j <= qloc).astype(BF16)
    return ident, rotm, onesc, onesr, maskm


def _swz(w):
    """[D, C] -> [128, NKT, C] with w[kt*128+p, c] at [p, kt, c]."""
    c = w.shape[1]
    return np.ascontiguousarray(
        w.reshape(NKT, 128, c).transpose(1, 0, 2)).astype(BF16)


def kernel(x, mask, cos, sin, Wq, Wk, Wv, Wo):
    from concourse.bass_utils import run_bass_kernel_spmd

    if "nc" not in _CACHE:
        _CACHE["nc"] = _build_program()
    nc = _CACHE["nc"]

    x = np.asarray(x, dtype=np.float32)
    cos = np.asarray(cos, dtype=np.float32)
    sin = np.asarray(sin, dtype=np.float32)
    Wq = np.asarray(Wq, dtype=np.float32)
    Wk = np.asarray(Wk, dtype=np.float32)
    Wv = np.asarray(Wv, dtype=np.float32)
    Wo = np.asarray(Wo, dtype=np.float32)

    xT = _swz(np.ascontiguousarray(x[0].T))          # [128, NKT, S]
    cosT = np.ascontiguousarray(cos.T)               # [DH, S]
    sinT = np.ascontiguousarray(sin.T)
    ident, rotm, onesc, onesr, maskm = _host_consts()

    in_maps = []
    for i in range(N_CORES):
        in_maps.append({
            "xT": xT,
            "wq": _swz(Wq[:, i * FPC:(i + 1) * FPC]),
            "wk": _swz(Wk[:, i * DH:(i + 1) * DH]),
            "wv": _swz(Wv[:, i * DH:(i + 1) * DH]),
            "wo": np.ascontiguousarray(Wo[i * FPC:(i + 1) * FPC, :]).astype(BF16),
            "cosT": cosT,
            "sinT": sinT,
            "maskm": maskm,
            "ident": ident,
            "rotm": rotm,
            "onesc": onesc,
            "onesr": onesr,
        })

    import os
    trace = bool(os.environ.get("BASS_TRACE"))
    res = run_bass_kernel_spmd(nc, in_maps, list(range(N_CORES)), trace=trace)
    _CACHE["last_exec_time_ns"] = res.exec_time_ns
    _CACHE["last_result"] = res

    out = np.zeros((S, D), dtype=np.float32)
    for i in range(N_CORES):
        out += res.results[i]["out"].astype(np.float32)
    return out[None]
